# revision 30
# baseline (speedup 1.0000x reference)
"""Trainium2 Bass kernel for a 2-layer SimpleRNN classifier.

Model (per reference):
  x = emb[tokens]                               # [B,T,E]
  seq1 = SimpleRNN_relu(x;  W1x, W1h, b1)       # [B,T,H1], return_sequences
  h    = SimpleRNN_relu(seq1; W2x, W2h, b2)[-1] # [B,H2], last step
  h = relu(h@Wd1+bd1); h = relu(h@Wd2+bd2); out = sigmoid(h@Wc+bc)  # [B,1]

Sharding: data-parallel over batch, 8 rows per core on 8 NeuronCores.
All activations are kept *transposed* on-chip (features on partitions,
(time, batch) on the free dim) so:
  - the recurrent state needs no per-step transpose,
  - weights are the PE stationary operand (fp16 -> fast weight load),
  - biases are per-partition vectors fused into ScalarE activations.

Compute dtype: fp16 operands with fp32 PSUM accumulation and fp32
xw (input-projection) buffers.
"""

import numpy as np

import concourse.bass as bass
import concourse.mybir as mybir
import concourse.tile as tile
from concourse.vector_clock import ScopedClock, VectorClock
from concourse.bass_utils import run_bass_kernel_spmd

# ---------------------------------------------------------------------------
# Problem constants (hardcoded per the task contract).
B, T, V, E = 64, 512, 50000, 300
H1, H2, D1, D2, C = 256, 512, 128, 64, 1
N_CORES = 8
BPC = B // N_CORES          # batch rows per core = 8
NT = T * BPC                # columns of the transposed activation = 4096
EP = 384                    # E padded to 3 partition chunks
KE, K1, K2 = EP // 128, H1 // 128, H2 // 128   # 3, 2, 4
BLK = 32                    # time steps per pipeline block
NBLK = T // BLK             # 8
NCOL_BLK = BLK * BPC        # 512 activation columns per block
GATH = NT // 128            # 32 gather tiles of 128 tokens

F16 = mybir.dt.float16
F32 = mybir.dt.float32
I32 = mybir.dt.int32
AF = mybir.ActivationFunctionType


MAX_WAITS = 1  # walrus in this container rejects more sem waits per inst


def _split_excess_waits(nc, max_waits=MAX_WAITS):
    """The container's walrus codegen rejects instructions carrying more than
    a couple of sem waits ("Too many sync wait commands"). Tile freely attaches
    many. Post-process the scheduled BIR: move excess waits onto injected NoOps
    placed immediately before the instruction on the same engine (engines
    process waits in instruction order, so semantics are preserved)."""
    ctr = 0
    for f in nc.m.functions:
        for b in f.blocks:
            new_insts = []
            changed = False
            for inst in b.instructions:
                s = inst.sync_info
                if s is not None and s.on_wait and len(s.on_wait) > max_waits:
                    w = list(s.on_wait)
                    n_extra = len(w) - max_waits
                    for i in range(0, n_extra, max_waits):
                        chunk = w[i : min(i + max_waits, n_extra)]
                        nop = mybir.InstNoOp(
                            name=f"bass_waitsplit_{ctr}",
                            engine=inst.engine,
                            ins=[],
                            outs=[],
                            sync_info=mybir.SyncInfo(on_wait=chunk, on_update=[]),
                        )
                        ctr += 1
                        new_insts.append(nop)
                    s.on_wait = w[n_extra:]
                    changed = True
                new_insts.append(inst)
            if changed:
                b.instructions = new_insts
    return ctr


def build_nc(t_steps=T):
    """Emit the per-core Bass program. t_steps<T builds a truncated model
    (debug only)."""
    nblk = t_steps // BLK
    nt = t_steps * BPC
    gath_tiles = nt // 128

    nc = bass.Bass()
    # ---- DRAM I/O (per core) ----
    tok_d = nc.dram_tensor("tokens", [128, gath_tiles], I32, kind="ExternalInput")
    emb_d = nc.dram_tensor("emb", [V, EP], F16, kind="ExternalInput")
    w1x_d = nc.dram_tensor("w1x", [128, KE, K1, 128], F16, kind="ExternalInput")
    w1h_d = nc.dram_tensor("w1h", [128, K1, K1, 128], F16, kind="ExternalInput")
    b1_d = nc.dram_tensor("b1", [128, K1], F32, kind="ExternalInput")
    w2x_d = nc.dram_tensor("w2x", [128, K1, K2, 128], F16, kind="ExternalInput")
    b2_d = nc.dram_tensor("b2", [128, K2], F32, kind="ExternalInput")
    w2h_d = nc.dram_tensor("w2h", [128, K2, K2, 128], F16, kind="ExternalInput")
    wd1_d = nc.dram_tensor("wd1", [128, K2, D1], F16, kind="ExternalInput")
    bd1_d = nc.dram_tensor("bd1", [D1, 1], F32, kind="ExternalInput")
    wd2_d = nc.dram_tensor("wd2", [D1, D2], F16, kind="ExternalInput")
    bd2_d = nc.dram_tensor("bd2", [D2, 1], F32, kind="ExternalInput")
    wc_d = nc.dram_tensor("wc", [D2, C], F16, kind="ExternalInput")
    bc_d = nc.dram_tensor("bc", [C, 1], F32, kind="ExternalInput")
    ident_d = nc.dram_tensor("ident", [128, 128], F16, kind="ExternalInput")
    out_d = nc.dram_tensor("out", [C, BPC], F32, kind="ExternalOutput")

    with tile.TileContext(nc) as tc:
        with (
            tc.tile_pool(name="const", bufs=1) as cpool,
            tc.tile_pool(name="act", bufs=1) as apool,
            tc.tile_pool(name="gath", bufs=4) as gpool,
            tc.tile_pool(name="xt", bufs=3) as xtpool,
            tc.tile_pool(name="tmp", bufs=8) as tpool,
            tc.tile_pool(name="psb", bufs=2, space="PSUM") as psb,
            tc.tile_pool(name="ps1", bufs=2, space="PSUM") as ps1,
            tc.tile_pool(name="ps2", bufs=2, space="PSUM") as ps2,
        ):
            # ---- load constants (weights/biases/tokens) ----
            def load(dram, shape, dtype):
                t = cpool.tile(shape, dtype, tag=dram.name)
                nc.sync.dma_start(out=t[:], in_=dram[:])
                return t

            tok_sb = load(tok_d, [128, gath_tiles], I32)
            w1x_sb = load(w1x_d, [128, KE, K1, 128], F16)
            w1h_sb = load(w1h_d, [128, K1, K1, 128], F16)
            b1_sb = load(b1_d, [128, K1], F32)
            w2x_sb = load(w2x_d, [128, K1, K2, 128], F16)
            b2_sb = load(b2_d, [128, K2], F32)
            w2h_sb = load(w2h_d, [128, K2, K2, 128], F16)
            wd1_sb = load(wd1_d, [128, K2, D1], F16)
            bd1_sb = load(bd1_d, [D1, 1], F32)
            wd2_sb = load(wd2_d, [D1, D2], F16)
            bd2_sb = load(bd2_d, [D2, 1], F32)
            wc_sb = load(wc_d, [D2, C], F16)
            bc_sb = load(bc_d, [C, 1], F32)
            ident_sb = load(ident_d, [128, 128], F16)

            # ---- persistent activation buffers (transposed layouts) ----
            # xw1T / xw2T: [feat_chunk, chunk, (t,b)] fp16
            xw1t = apool.tile([128, K1, nt], F16, tag="xw1t")
            xw2t = apool.tile([128, K2, nt], F16, tag="xw2t")
            # seq1T doubles as RNN1 state history; col 0:8 is h0=0,
            # step t writes cols 8+8t : 16+8t.
            seq1t = apool.tile([128, K1, nt + BPC], F16, tag="seq1t")
            # RNN2 state ping-pong: cols 0:8 zeros, slots at 8:16, 16:24.
            h2t = apool.tile([128, K2, 3 * BPC], F16, tag="h2t")
            out_sb = apool.tile([C, BPC], F32, tag="out_sb")

            nc.vector.memzero(seq1t[:, :, 0:BPC])
            nc.vector.memzero(h2t[:])

            # ---- embedding gather + transpose, per 128-token tile ----
            # xT block tiles [128, KE, 512] fp16 feed the xw1 bulk matmul.
            def bulk_proj_cols(w_sb, n_k, n_m, rhs_sl, bias_sb, dst, dst_col,
                               ncol):
                """dst[:, m, dst_col:+ncol] = W.T @ rhs + bias."""
                for m in range(n_m):
                    ps = psb.tile([128, ncol], F32, tag="psb")
                    for k in range(n_k):
                        nc.tensor.matmul(
                            out=ps[:],
                            lhsT=w_sb[:, k, m, :],
                            rhs=rhs_sl[:, k, :],
                            start=(k == 0),
                            stop=(k == n_k - 1),
                        )
                    nc.scalar.activation(
                        out=dst[:, m, dst_col : dst_col + ncol],
                        in_=ps[:],
                        func=AF.Identity,
                        bias=bias_sb[:, m : m + 1],
                        scale=1.0,
                    )

            xt_tiles = []
            gpb = NCOL_BLK // 128  # gather tiles per block
            tr_engines = [nc.sync, nc.scalar]
            for blk in range(nblk):
                xt = xtpool.tile([128, KE, NCOL_BLK], F16, tag="xt")
                xt_tiles.append(xt)
                for gi in range(gpb):
                    g = blk * gpb + gi
                    gt = gpool.tile([128, EP], F16, tag="gt")
                    nc.gpsimd.indirect_dma_start(
                        out=gt[:],
                        out_offset=None,
                        in_=emb_d[:],
                        in_offset=bass.IndirectOffsetOnAxis(
                            ap=tok_sb[:, g : g + 1], axis=0
                        ),
                    )
                    for c in range(KE):
                        tr_engines[(g * KE + c) % 2].dma_start(
                            out=xt[:, c, gi * 128 : (gi + 1) * 128],
                            in_=gt[:, c * 128 : (c + 1) * 128],
                            transpose=True,
                        )
                    if blk == 0:
                        # make block-0 xw1 available per gather tile so the
                        # rnn pipeline starts ~8us earlier
                        bulk_proj_cols(
                            w1x_sb, KE, K1, xt[:, :, gi * 128 : (gi + 1) * 128],
                            b1_sb, xw1t, gi * 128, 128,
                        )

            # ---- helpers ----
            def bulk_proj(w_sb, n_k, n_m, rhs_sl, bias_sb, dst, dst_col):
                bulk_proj_cols(w_sb, n_k, n_m, rhs_sl, bias_sb, dst, dst_col,
                               NCOL_BLK)

            def rnn_step(w_sb, n_k, xwt, state_sl, dst_sl, pspool, halves):
                """dst = relu(xw_t + Wh.T @ state). xw_t enters PSUM via one
                identity matmul per half (covering all its regions), then the
                Wh chunks accumulate; the relu is split into halves on separate
                PSUM banks so the first half overlaps the second's matmuls."""
                for m_lo, m_hi, pstag in halves:
                    nh = m_hi - m_lo
                    ps = pspool.tile([128, nh, BPC], F32, tag=pstag)
                    nc.tensor.matmul(
                        out=ps[:],
                        lhsT=ident_sb[:],
                        rhs=xwt[:, m_lo:m_hi, :],
                        start=True,
                        stop=False,
                        skip_group_check=True,
                    )
                    # k-interleaved across the half's m-regions so matmuls
                    # consuming the freshest state chunks come as late as
                    # possible (hides the previous step's relu latency)
                    for k in range(n_k):
                        for m in range(m_lo, m_hi):
                            nc.tensor.matmul(
                                out=ps[:, m - m_lo, :],
                                lhsT=w_sb[:, k, m, :],
                                rhs=state_sl[:, k, :],
                                start=False,
                                stop=(k == n_k - 1),
                                skip_group_check=True,
                            )
                    nc.vector.tensor_relu(dst_sl[:, m_lo:m_hi, :], ps[:])

            def rnn1_step(t):
                rnn_step(
                    w1h_sb, K1,
                    xw1t[:, :, t * BPC : (t + 1) * BPC],
                    seq1t[:, :, t * BPC : (t + 1) * BPC],
                    seq1t[:, :, (t + 1) * BPC : (t + 2) * BPC],
                    ps1, [(0, K1, "r1")],
                )

            def rnn2_step(t):
                src = 0 if t == 0 else (BPC + ((t - 1) % 2) * BPC)
                dst = BPC + (t % 2) * BPC
                rnn_step(
                    w2h_sb, K2,
                    xw2t[:, :, t * BPC : (t + 1) * BPC],
                    h2t[:, :, src : src + BPC],
                    h2t[:, :, dst : dst + BPC],
                    ps2, [(0, 2, "r2a"), (2, 4, "r2b")],
                )

            # ---- main pipeline: layer-2 runs SKEW steps behind layer-1 ----
            # SKEW > BLK so block-boundary bulk work (xw2 matmuls + bias
            # copies) never gates the immediately following rnn2 steps.
            SKEW = BLK + 4

            def bulk_xw1(b):
                bulk_proj(w1x_sb, KE, K1, xt_tiles[b][:], b1_sb, xw1t,
                          b * NCOL_BLK)

            def bulk_xw2(b):
                bulk_proj(
                    w2x_sb, K1, K2,
                    seq1t[:, :, BPC + b * NCOL_BLK : BPC + (b + 1) * NCOL_BLK],
                    b2_sb, xw2t, b * NCOL_BLK,
                )

            # (block-0 xw1 was emitted per gather tile above)
            for blk in range(nblk):
                for ti in range(BLK):
                    if ti == BLK // 2 and blk + 1 < nblk:
                        bulk_xw1(blk + 1)
                    # rnn2 first: its relus are the critical chain and must
                    # not queue behind rnn1's relu on the in-order DVE
                    t2 = blk * BLK + ti - SKEW
                    if t2 >= 0:
                        rnn2_step(t2)
                    rnn1_step(blk * BLK + ti)
                bulk_xw2(blk)
            for t2 in range(max(0, nblk * BLK - SKEW), t_steps):
                rnn2_step(t2)

            # ---- dense head on the final RNN2 state ----
            t_last = t_steps - 1
            hfin = h2t[:, :, BPC + (t_last % 2) * BPC : 2 * BPC + (t_last % 2) * BPC]

            ps = ps1.tile([D1, BPC], F32, tag="r1")
            for k in range(K2):
                nc.tensor.matmul(out=ps[:], lhsT=wd1_sb[:, k, :], rhs=hfin[:, k, :],
                                 start=(k == 0), stop=(k == K2 - 1))
            d1 = tpool.tile([D1, BPC], F16, tag="d1")
            nc.scalar.activation(out=d1[:], in_=ps[:], func=AF.Relu,
                                 bias=bd1_sb[:, 0:1], scale=1.0)

            ps = ps1.tile([D2, BPC], F32, tag="r1")
            nc.tensor.matmul(out=ps[:], lhsT=wd2_sb[:], rhs=d1[:], start=True,
                             stop=True)
            d2 = tpool.tile([D2, BPC], F16, tag="d2")
            nc.scalar.activation(out=d2[:], in_=ps[:], func=AF.Relu,
                                 bias=bd2_sb[:, 0:1], scale=1.0)

            ps = ps1.tile([C, BPC], F32, tag="r1")
            nc.tensor.matmul(out=ps[:], lhsT=wc_sb[:], rhs=d2[:], start=True,
                             stop=True)
            nc.scalar.activation(out=out_sb[:], in_=ps[:], func=AF.Sigmoid,
                                 bias=bc_sb[:, 0:1], scale=1.0)
            nc.sync.dma_start(out=out_d[:], in_=out_sb[:])

    n_split = _split_excess_waits(nc)
    print(f"[kernel] split {n_split} excess-wait NoOps")
    return nc


# ---------------------------------------------------------------------------
# Host-side input prep


def _chunk_lhsT(w, kc, mc):
    """[K, M] -> [128, kc, mc, 128] fp16 lhsT chunk layout."""
    K, M = w.shape
    assert K == kc * 128 and M == mc * 128
    return np.ascontiguousarray(
        w.reshape(kc, 128, mc, 128).transpose(1, 0, 2, 3)
    ).astype(np.float16)


def prep_core_inputs(inputs, t_steps=T):
    """Returns (shared_weight_map, per_core_token_list)."""
    emb = np.asarray(inputs["emb"], np.float32)
    emb_p = np.zeros((V, EP), np.float16)
    emb_p[:, :E] = emb.astype(np.float16)

    w1x = np.zeros((EP, H1), np.float32)
    w1x[:E] = np.asarray(inputs["W1x"], np.float32)

    shared = {
        "emb": emb_p,
        "w1x": _chunk_lhsT(w1x, KE, K1),
        "w1h": _chunk_lhsT(np.asarray(inputs["W1h"], np.float32), K1, K1),
        "b1": np.ascontiguousarray(
            np.asarray(inputs["b1"], np.float32).reshape(K1, 128).T
        ),
        "w2x": _chunk_lhsT(np.asarray(inputs["W2x"], np.float32), K1, K2),
        "b2": np.ascontiguousarray(
            np.asarray(inputs["b2"], np.float32).reshape(K2, 128).T
        ),
        "w2h": _chunk_lhsT(np.asarray(inputs["W2h"], np.float32), K2, K2),
        "wd1": np.ascontiguousarray(
            np.asarray(inputs["Wd1"], np.float32).reshape(K2, 128, D1)
            .transpose(1, 0, 2)
        ).astype(np.float16),
        "bd1": np.asarray(inputs["bd1"], np.float32).reshape(D1, 1),
        "wd2": np.asarray(inputs["Wd2"], np.float32).astype(np.float16),
        "bd2": np.asarray(inputs["bd2"], np.float32).reshape(D2, 1),
        "wc": np.asarray(inputs["Wc"], np.float32).astype(np.float16),
        "bc": np.asarray(inputs["bc"], np.float32).reshape(C, 1),
        "ident": np.eye(128, dtype=np.float16),
    }

    tokens = np.asarray(inputs["tokens"], np.int32)
    per_core_tok = []
    gath_tiles = (t_steps * BPC) // 128
    for c in range(N_CORES):
        cols = tokens[c * BPC : (c + 1) * BPC, :t_steps].T.reshape(-1)  # (t,b)
        per_core_tok.append(
            np.ascontiguousarray(cols.reshape(gath_tiles, 128).T)
        )
    return shared, per_core_tok


_CACHE = {}


def run(inputs, t_steps=T, trace=False):
    key = t_steps
    if key not in _CACHE:
        _CACHE[key] = build_nc(t_steps)
    nc = _CACHE[key]
    shared, per_core_tok = prep_core_inputs(inputs, t_steps)
    in_maps = [dict(shared, tokens=per_core_tok[c]) for c in range(N_CORES)]
    res = run_bass_kernel_spmd(
        nc, in_maps, core_ids=list(range(N_CORES)), trace=trace
    )
    out = np.concatenate(
        [res.results[c]["out"].reshape(BPC, C) for c in range(N_CORES)], axis=0
    )
    return out.astype(np.float32), res


def kernel(**inputs):
    out, _ = run(inputs)
    return out


# revision 33
# speedup vs baseline: 1.0075x; 1.0075x over previous
"""Trainium2 Bass kernel for a 2-layer SimpleRNN classifier.

Model (per reference):
  x = emb[tokens]                               # [B,T,E]
  seq1 = SimpleRNN_relu(x;  W1x, W1h, b1)       # [B,T,H1], return_sequences
  h    = SimpleRNN_relu(seq1; W2x, W2h, b2)[-1] # [B,H2], last step
  h = relu(h@Wd1+bd1); h = relu(h@Wd2+bd2); out = sigmoid(h@Wc+bc)  # [B,1]

Sharding: data-parallel over batch, 8 rows per core on 8 NeuronCores.
All activations are kept *transposed* on-chip (features on partitions,
(time, batch) on the free dim) so:
  - the recurrent state needs no per-step transpose,
  - weights are the PE stationary operand (fp16 -> fast weight load),
  - biases are per-partition vectors fused into ScalarE activations.

Compute dtype: fp16 operands with fp32 PSUM accumulation and fp32
xw (input-projection) buffers.
"""

import numpy as np

import concourse.bass as bass
import concourse.mybir as mybir
import concourse.tile as tile
from concourse.vector_clock import ScopedClock, VectorClock
from concourse.bass_utils import run_bass_kernel_spmd

# ---------------------------------------------------------------------------
# Problem constants (hardcoded per the task contract).
B, T, V, E = 64, 512, 50000, 300
H1, H2, D1, D2, C = 256, 512, 128, 64, 1
N_CORES = 8
BPC = B // N_CORES          # batch rows per core = 8
NT = T * BPC                # columns of the transposed activation = 4096
EP = 384                    # E padded to 3 partition chunks
KE, K1, K2 = EP // 128, H1 // 128, H2 // 128   # 3, 2, 4
BLK = 32                    # time steps per pipeline block
NBLK = T // BLK             # 8
NCOL_BLK = BLK * BPC        # 512 activation columns per block
GATH = NT // 128            # 32 gather tiles of 128 tokens

F16 = mybir.dt.float16
F32 = mybir.dt.float32
I32 = mybir.dt.int32
AF = mybir.ActivationFunctionType


MAX_WAITS = 1  # walrus in this container rejects more sem waits per inst


def _split_excess_waits(nc, max_waits=MAX_WAITS):
    """The container's walrus codegen rejects instructions carrying more than
    a couple of sem waits ("Too many sync wait commands"). Tile freely attaches
    many. Post-process the scheduled BIR: move excess waits onto injected NoOps
    placed immediately before the instruction on the same engine (engines
    process waits in instruction order, so semantics are preserved)."""
    ctr = 0
    for f in nc.m.functions:
        for b in f.blocks:
            new_insts = []
            changed = False
            for inst in b.instructions:
                s = inst.sync_info
                if s is not None and s.on_wait and len(s.on_wait) > max_waits:
                    w = list(s.on_wait)
                    n_extra = len(w) - max_waits
                    for i in range(0, n_extra, max_waits):
                        chunk = w[i : min(i + max_waits, n_extra)]
                        nop = mybir.InstNoOp(
                            name=f"bass_waitsplit_{ctr}",
                            engine=inst.engine,
                            ins=[],
                            outs=[],
                            sync_info=mybir.SyncInfo(on_wait=chunk, on_update=[]),
                        )
                        ctr += 1
                        new_insts.append(nop)
                    s.on_wait = w[n_extra:]
                    changed = True
                new_insts.append(inst)
            if changed:
                b.instructions = new_insts
    return ctr


def build_nc(t_steps=T):
    """Emit the per-core Bass program. t_steps<T builds a truncated model
    (debug only)."""
    nblk = t_steps // BLK
    nt = t_steps * BPC
    gath_tiles = nt // 128

    nc = bass.Bass()
    # ---- DRAM I/O (per core) ----
    tok_d = nc.dram_tensor("tokens", [128, gath_tiles], I32, kind="ExternalInput")
    emb_d = nc.dram_tensor("emb", [V, EP], F16, kind="ExternalInput")
    w1x_d = nc.dram_tensor("w1x", [128, KE, K1, 128], F16, kind="ExternalInput")
    w1h_d = nc.dram_tensor("w1h", [128, K1, K1, 128], F16, kind="ExternalInput")
    b1_d = nc.dram_tensor("b1", [128, K1], F32, kind="ExternalInput")
    w2x_d = nc.dram_tensor("w2x", [128, K1, K2, 128], F16, kind="ExternalInput")
    b2_d = nc.dram_tensor("b2", [128, K2], F32, kind="ExternalInput")
    w2h_d = nc.dram_tensor("w2h", [128, K2, K2, 128], F16, kind="ExternalInput")
    wd1_d = nc.dram_tensor("wd1", [128, K2, D1], F16, kind="ExternalInput")
    bd1_d = nc.dram_tensor("bd1", [D1, 1], F32, kind="ExternalInput")
    wd2_d = nc.dram_tensor("wd2", [D1, D2], F16, kind="ExternalInput")
    bd2_d = nc.dram_tensor("bd2", [D2, 1], F32, kind="ExternalInput")
    wc_d = nc.dram_tensor("wc", [D2, C], F16, kind="ExternalInput")
    bc_d = nc.dram_tensor("bc", [C, 1], F32, kind="ExternalInput")
    ident_d = nc.dram_tensor("ident", [128, 128], F16, kind="ExternalInput")
    out_d = nc.dram_tensor("out", [C, BPC], F32, kind="ExternalOutput")

    with tile.TileContext(nc) as tc:
        with (
            tc.tile_pool(name="const", bufs=1) as cpool,
            tc.tile_pool(name="act", bufs=1) as apool,
            tc.tile_pool(name="gath", bufs=4) as gpool,
            tc.tile_pool(name="xt", bufs=3) as xtpool,
            tc.tile_pool(name="tmp", bufs=8) as tpool,
            tc.tile_pool(name="psb", bufs=2, space="PSUM") as psb,
            tc.tile_pool(name="ps1", bufs=2, space="PSUM") as ps1,
            tc.tile_pool(name="ps2", bufs=2, space="PSUM") as ps2,
        ):
            # ---- load constants (weights/biases/tokens) ----
            def load(dram, shape, dtype):
                t = cpool.tile(shape, dtype, tag=dram.name)
                nc.sync.dma_start(out=t[:], in_=dram[:])
                return t

            tok_sb = load(tok_d, [128, gath_tiles], I32)
            w1x_sb = load(w1x_d, [128, KE, K1, 128], F16)
            w1h_sb = load(w1h_d, [128, K1, K1, 128], F16)
            b1_sb = load(b1_d, [128, K1], F32)
            w2x_sb = load(w2x_d, [128, K1, K2, 128], F16)
            b2_sb = load(b2_d, [128, K2], F32)
            w2h_sb = load(w2h_d, [128, K2, K2, 128], F16)
            wd1_sb = load(wd1_d, [128, K2, D1], F16)
            bd1_sb = load(bd1_d, [D1, 1], F32)
            wd2_sb = load(wd2_d, [D1, D2], F16)
            bd2_sb = load(bd2_d, [D2, 1], F32)
            wc_sb = load(wc_d, [D2, C], F16)
            bc_sb = load(bc_d, [C, 1], F32)
            ident_sb = load(ident_d, [128, 128], F16)

            # ---- persistent activation buffers (transposed layouts) ----
            # xw1T / xw2T: [feat_chunk, chunk, (t,b)] fp16
            xw1t = apool.tile([128, K1, nt], F16, tag="xw1t")
            xw2t = apool.tile([128, K2, nt], F16, tag="xw2t")
            # seq1T doubles as RNN1 state history; col 0:8 is h0=0,
            # step t writes cols 8+8t : 16+8t.
            seq1t = apool.tile([128, K1, nt + BPC], F16, tag="seq1t")
            # RNN2 state ping-pong: cols 0:8 zeros, slots at 8:16, 16:24.
            h2t = apool.tile([128, K2, 3 * BPC], F16, tag="h2t")
            out_sb = apool.tile([C, BPC], F32, tag="out_sb")

            nc.vector.memzero(seq1t[:, :, 0:BPC])
            nc.vector.memzero(h2t[:])

            # ---- embedding gather + transpose, per 128-token tile ----
            # xT block tiles [128, KE, 512] fp16 feed the xw1 bulk matmul.
            def bulk_proj_cols(w_sb, n_k, n_m, rhs_sl, bias_sb, dst, dst_col,
                               ncol):
                """dst[:, m, dst_col:+ncol] = W.T @ rhs + bias."""
                for m in range(n_m):
                    ps = psb.tile([128, ncol], F32, tag="psb")
                    for k in range(n_k):
                        nc.tensor.matmul(
                            out=ps[:],
                            lhsT=w_sb[:, k, m, :],
                            rhs=rhs_sl[:, k, :],
                            start=(k == 0),
                            stop=(k == n_k - 1),
                        )
                    nc.scalar.activation(
                        out=dst[:, m, dst_col : dst_col + ncol],
                        in_=ps[:],
                        func=AF.Identity,
                        bias=bias_sb[:, m : m + 1],
                        scale=1.0,
                    )

            xt_tiles = []
            gpb = NCOL_BLK // 128  # gather tiles per block
            tr_engines = [nc.sync, nc.scalar]
            for blk in range(nblk):
                xt = xtpool.tile([128, KE, NCOL_BLK], F16, tag="xt")
                xt_tiles.append(xt)
                for gi in range(gpb):
                    g = blk * gpb + gi
                    gt = gpool.tile([128, EP], F16, tag="gt")
                    nc.gpsimd.indirect_dma_start(
                        out=gt[:],
                        out_offset=None,
                        in_=emb_d[:],
                        in_offset=bass.IndirectOffsetOnAxis(
                            ap=tok_sb[:, g : g + 1], axis=0
                        ),
                    )
                    for c in range(KE):
                        tr_engines[(g * KE + c) % 2].dma_start(
                            out=xt[:, c, gi * 128 : (gi + 1) * 128],
                            in_=gt[:, c * 128 : (c + 1) * 128],
                            transpose=True,
                        )
                    if blk == 0:
                        # make block-0 xw1 available per gather tile so the
                        # rnn pipeline starts ~8us earlier
                        bulk_proj_cols(
                            w1x_sb, KE, K1, xt[:, :, gi * 128 : (gi + 1) * 128],
                            b1_sb, xw1t, gi * 128, 128,
                        )

            # ---- helpers ----
            def bulk_proj(w_sb, n_k, n_m, rhs_sl, bias_sb, dst, dst_col):
                bulk_proj_cols(w_sb, n_k, n_m, rhs_sl, bias_sb, dst, dst_col,
                               NCOL_BLK)

            def rnn_step(w_sb, n_k, xwt, state_sl, dst_sl, pspool, halves,
                         relu_prio=None):
                """dst = relu(xw_t + Wh.T @ state). xw_t enters PSUM via one
                identity matmul per half (covering all its regions), then the
                Wh chunks accumulate; the relu is split into halves on separate
                PSUM banks so the first half overlaps the second's matmuls."""
                for m_lo, m_hi, pstag in halves:
                    nh = m_hi - m_lo
                    ps = pspool.tile([128, nh, BPC], F32, tag=pstag)
                    nc.tensor.matmul(
                        out=ps[:],
                        lhsT=ident_sb[:],
                        rhs=xwt[:, m_lo:m_hi, :],
                        start=True,
                        stop=False,
                        skip_group_check=True,
                    )
                    # k-interleaved across the half's m-regions so matmuls
                    # consuming the freshest state chunks come as late as
                    # possible (hides the previous step's relu latency)
                    for k in range(n_k):
                        for m in range(m_lo, m_hi):
                            nc.tensor.matmul(
                                out=ps[:, m - m_lo, :],
                                lhsT=w_sb[:, k, m, :],
                                rhs=state_sl[:, k, :],
                                start=False,
                                stop=(k == n_k - 1),
                                skip_group_check=True,
                            )
                    if relu_prio is not None:
                        with tc.high_priority(relu_prio):
                            nc.vector.tensor_relu(dst_sl[:, m_lo:m_hi, :], ps[:])
                    else:
                        nc.vector.tensor_relu(dst_sl[:, m_lo:m_hi, :], ps[:])

            def rnn1_step(t):
                rnn_step(
                    w1h_sb, K1,
                    xw1t[:, :, t * BPC : (t + 1) * BPC],
                    seq1t[:, :, t * BPC : (t + 1) * BPC],
                    seq1t[:, :, (t + 1) * BPC : (t + 2) * BPC],
                    ps1, [(0, K1, "r1")],
                )

            def rnn2_step(t):
                src = 0 if t == 0 else (BPC + ((t - 1) % 2) * BPC)
                dst = BPC + (t % 2) * BPC
                rnn_step(
                    w2h_sb, K2,
                    xw2t[:, :, t * BPC : (t + 1) * BPC],
                    h2t[:, :, src : src + BPC],
                    h2t[:, :, dst : dst + BPC],
                    ps2, [(0, 2, "r2a"), (2, 4, "r2b")],
                    relu_prio=40,
                )

            # ---- main pipeline: layer-2 runs SKEW steps behind layer-1 ----
            # SKEW > BLK so block-boundary bulk work (xw2 matmuls + bias
            # copies) never gates the immediately following rnn2 steps.
            SKEW = BLK + 4

            def bulk_xw1(b):
                bulk_proj(w1x_sb, KE, K1, xt_tiles[b][:], b1_sb, xw1t,
                          b * NCOL_BLK)

            def bulk_xw2(b):
                bulk_proj(
                    w2x_sb, K1, K2,
                    seq1t[:, :, BPC + b * NCOL_BLK : BPC + (b + 1) * NCOL_BLK],
                    b2_sb, xw2t, b * NCOL_BLK,
                )

            # (block-0 xw1 was emitted per gather tile above)
            for blk in range(nblk):
                for ti in range(BLK):
                    if ti == BLK // 2 and blk + 1 < nblk:
                        bulk_xw1(blk + 1)
                    # rnn2 first: its relus are the critical chain and must
                    # not queue behind rnn1's relu on the in-order DVE
                    t2 = blk * BLK + ti - SKEW
                    if t2 >= 0:
                        rnn2_step(t2)
                    rnn1_step(blk * BLK + ti)
                bulk_xw2(blk)
            for t2 in range(max(0, nblk * BLK - SKEW), t_steps):
                rnn2_step(t2)

            # ---- dense head on the final RNN2 state ----
            t_last = t_steps - 1
            hfin = h2t[:, :, BPC + (t_last % 2) * BPC : 2 * BPC + (t_last % 2) * BPC]

            ps = ps1.tile([D1, BPC], F32, tag="r1")
            for k in range(K2):
                nc.tensor.matmul(out=ps[:], lhsT=wd1_sb[:, k, :], rhs=hfin[:, k, :],
                                 start=(k == 0), stop=(k == K2 - 1))
            d1 = tpool.tile([D1, BPC], F16, tag="d1")
            nc.scalar.activation(out=d1[:], in_=ps[:], func=AF.Relu,
                                 bias=bd1_sb[:, 0:1], scale=1.0)

            ps = ps1.tile([D2, BPC], F32, tag="r1")
            nc.tensor.matmul(out=ps[:], lhsT=wd2_sb[:], rhs=d1[:], start=True,
                             stop=True)
            d2 = tpool.tile([D2, BPC], F16, tag="d2")
            nc.scalar.activation(out=d2[:], in_=ps[:], func=AF.Relu,
                                 bias=bd2_sb[:, 0:1], scale=1.0)

            ps = ps1.tile([C, BPC], F32, tag="r1")
            nc.tensor.matmul(out=ps[:], lhsT=wc_sb[:], rhs=d2[:], start=True,
                             stop=True)
            nc.scalar.activation(out=out_sb[:], in_=ps[:], func=AF.Sigmoid,
                                 bias=bc_sb[:, 0:1], scale=1.0)
            nc.sync.dma_start(out=out_d[:], in_=out_sb[:])

    n_split = _split_excess_waits(nc)
    print(f"[kernel] split {n_split} excess-wait NoOps")
    return nc


# ---------------------------------------------------------------------------
# Host-side input prep


def _chunk_lhsT(w, kc, mc):
    """[K, M] -> [128, kc, mc, 128] fp16 lhsT chunk layout."""
    K, M = w.shape
    assert K == kc * 128 and M == mc * 128
    return np.ascontiguousarray(
        w.reshape(kc, 128, mc, 128).transpose(1, 0, 2, 3)
    ).astype(np.float16)


def prep_core_inputs(inputs, t_steps=T):
    """Returns (shared_weight_map, per_core_token_list)."""
    emb = np.asarray(inputs["emb"], np.float32)
    emb_p = np.zeros((V, EP), np.float16)
    emb_p[:, :E] = emb.astype(np.float16)

    w1x = np.zeros((EP, H1), np.float32)
    w1x[:E] = np.asarray(inputs["W1x"], np.float32)

    shared = {
        "emb": emb_p,
        "w1x": _chunk_lhsT(w1x, KE, K1),
        "w1h": _chunk_lhsT(np.asarray(inputs["W1h"], np.float32), K1, K1),
        "b1": np.ascontiguousarray(
            np.asarray(inputs["b1"], np.float32).reshape(K1, 128).T
        ),
        "w2x": _chunk_lhsT(np.asarray(inputs["W2x"], np.float32), K1, K2),
        "b2": np.ascontiguousarray(
            np.asarray(inputs["b2"], np.float32).reshape(K2, 128).T
        ),
        "w2h": _chunk_lhsT(np.asarray(inputs["W2h"], np.float32), K2, K2),
        "wd1": np.ascontiguousarray(
            np.asarray(inputs["Wd1"], np.float32).reshape(K2, 128, D1)
            .transpose(1, 0, 2)
        ).astype(np.float16),
        "bd1": np.asarray(inputs["bd1"], np.float32).reshape(D1, 1),
        "wd2": np.asarray(inputs["Wd2"], np.float32).astype(np.float16),
        "bd2": np.asarray(inputs["bd2"], np.float32).reshape(D2, 1),
        "wc": np.asarray(inputs["Wc"], np.float32).astype(np.float16),
        "bc": np.asarray(inputs["bc"], np.float32).reshape(C, 1),
        "ident": np.eye(128, dtype=np.float16),
    }

    tokens = np.asarray(inputs["tokens"], np.int32)
    per_core_tok = []
    gath_tiles = (t_steps * BPC) // 128
    for c in range(N_CORES):
        cols = tokens[c * BPC : (c + 1) * BPC, :t_steps].T.reshape(-1)  # (t,b)
        per_core_tok.append(
            np.ascontiguousarray(cols.reshape(gath_tiles, 128).T)
        )
    return shared, per_core_tok


_CACHE = {}


def run(inputs, t_steps=T, trace=False):
    key = t_steps
    if key not in _CACHE:
        _CACHE[key] = build_nc(t_steps)
    nc = _CACHE[key]
    shared, per_core_tok = prep_core_inputs(inputs, t_steps)
    in_maps = [dict(shared, tokens=per_core_tok[c]) for c in range(N_CORES)]
    res = run_bass_kernel_spmd(
        nc, in_maps, core_ids=list(range(N_CORES)), trace=trace
    )
    out = np.concatenate(
        [res.results[c]["out"].reshape(BPC, C) for c in range(N_CORES)], axis=0
    )
    return out.astype(np.float32), res


def kernel(**inputs):
    out, _ = run(inputs)
    return out
